# revision 26
# baseline (speedup 1.0000x reference)
"""DenseCaps dynamic-routing kernel for 8 Trainium2 NeuronCores.

Sharding: in_caps (I=8000) split 8 ways (1000/core, padded to 1024); the
per-iteration sum over in_caps is completed with a tiny AllReduce of the
partial s tensors ([64,10,16] fp32 = 40KB). Everything is SBUF-resident;
u_hat is never materialized:

  s[b,o,d]   = sum_{i,e} W[o,i,d,e] * (c[b,o,i] * x[b,i,e])   (K=i matmuls,
               moving operand y = c*x built on VectorE in i-partition layout)
  a[b,o,i]   = sum_e x[b,i,e] * g[b,o,i,e],
  g[b,o,i,e] = sum_d W[o,i,d,e] * v[b,o,d]                    (K=d matmuls)

Everything is fp32 (the correctness gate is elementwise-relative,
so flat bf16 noise in the big sums is not tolerable). ITERS=3.
"""

import sys
import numpy as np

sys.path.insert(0, "/opt/trn_rl_repo")

B, O, D, E = 64, 10, 16, 8
I_FULL = 8000
N_CORES = 8
IC = 1000          # in_caps per core
ICP = 1024         # padded
G = ICP // 128     # i-chunks of 128 partitions
ITERS = 3

_CACHE = {}


def _build_program():
    import concourse.bacc as bacc
    import concourse.mybir as mybir
    import concourse.tile as tile

    f32 = mybir.dt.float32
    bf16 = mybir.dt.bfloat16
    AF = mybir.ActivationFunctionType
    ALU = mybir.AluOpType
    AX = mybir.AxisListType

    nc = bacc.Bacc("TRN2", target_bir_lowering=False, debug=False,
                   num_devices=N_CORES)

    xi_d = nc.dram_tensor("xi", [128, G, E, B], f32, kind="ExternalInput")
    ws_d = nc.dram_tensor("ws", [128, G, E, O, D], f32, kind="ExternalInput")
    wg_d = nc.dram_tensor("wg", [G, 16, E, O, 128], f32, kind="ExternalInput")
    vout_d = nc.dram_tensor("vout", [16, O * B], f32, kind="ExternalOutput")
    import os
    DBG = bool(int(os.environ.get("KDBG", "0")))
    if DBG:
        ell_d = nc.dram_tensor("ell_dbg", [128, G, O, B], f32,
                               kind="ExternalOutput")

    with tile.TileContext(nc) as tc:
        with tc.tile_pool(name="const", bufs=1) as cpool, \
             tc.tile_pool(name="work", bufs=2) as wpool, \
             tc.tile_pool(name="small", bufs=1) as spool, \
             tc.tile_pool(name="ps_s", bufs=1, space="PSUM") as ps_s, \
             tc.tile_pool(name="ps_g", bufs=2, space="PSUM") as ps_g, \
             tc.tile_pool(name="ps_q", bufs=2, space="PSUM") as ps_q, \
             tc.tile_pool(name="dram", bufs=2, space="DRAM") as dpool:

            xi = cpool.tile([128, G, E, B], f32)
            ws = cpool.tile([128, G, E, O, D], f32)
            ell = cpool.tile([128, G, O, B], f32)
            ones16 = cpool.tile([16, 1], f32)

            nc.sync.dma_start(xi[:], xi_d.ap())
            nc.sync.dma_start(ws[:], ws_d.ap())
            nc.vector.memset(ell[:], 0.0)
            nc.vector.memset(ones16[:], 1.0)

            for k in range(ITERS):
                # ---------- routing weights c ----------
                if k > 0:
                    # c[b,o,i] = exp(ell)/sum_o exp(ell); |ell| << 1 so no
                    # max-subtraction is needed.
                    eexp = spool.tile([128, G, O, B], f32, tag="eexp")
                    nc.scalar.activation(eexp[:], ell[:], AF.Exp)
                    z = wpool.tile([128, G, B], f32, tag="z")
                    nc.vector.tensor_reduce(
                        z[:], eexp[:].transpose([0, 1, 3, 2]),
                        axis=AX.X, op=ALU.add)
                    nc.vector.reciprocal(z[:], z[:])
                    cc = eexp  # normalize in place: cc = eexp * (1/z)
                    nc.vector.tensor_mul(
                        cc[:], eexp[:],
                        z[:, :, None, :].broadcast_to([128, G, O, B]))

                # ---------- s partials: 640 K=128 matmuls ----------
                s_ps1 = ps_s.tile([16, 512], f32, tag="s1")
                s_ps2 = ps_s.tile([16, 128], f32, tag="s2")
                for g in range(G):
                    if k > 0:
                        y = wpool.tile([128, E, O, B], f32, tag="y")
                        nc.vector.tensor_mul(
                            y[:],
                            cc[:, g][:, None, :, :].broadcast_to([128, E, O, B]),
                            xi[:, g][:, :, None, :].broadcast_to([128, E, O, B]))
                    for e in range(E):
                        for o in range(O):
                            rhs = y[:, e, o, :] if k > 0 else xi[:, g, e, :]
                            out_ps = (s_ps1[:, o * 64:(o + 1) * 64] if o < 8
                                      else s_ps2[:, (o - 8) * 64:(o - 7) * 64])
                            # start clears has_written for the WHOLE bank, so
                            # only the first matmul touching each bank sets it.
                            nc.tensor.matmul(
                                out_ps, ws[:, g, e, o, :], rhs,
                                start=(g == 0 and e == 0 and o in (0, 8)),
                                stop=(g == G - 1 and e == E - 1))

                s_sb = spool.tile([16, O * B], f32, tag="ssb")
                nc.vector.tensor_copy(s_sb[:, 0:512], s_ps1[:])
                nc.vector.tensor_copy(s_sb[:, 512:640], s_ps2[:])
                if k == 0:
                    # c == 0.1 exactly on the first iteration
                    nc.vector.tensor_scalar(
                        out=s_sb[:], in0=s_sb[:], scalar1=0.1, scalar2=None,
                        op0=ALU.mult)

                # ---------- AllReduce over the i-shards ----------
                arin = dpool.tile([16, O * B], f32, tag="arin")
                arout = dpool.tile([16, O * B], f32, tag="arout")
                nc.sync.dma_start(arin[:], s_sb[:])
                nc.gpsimd.collective_compute(
                    "AllReduce", ALU.add,
                    replica_groups=[list(range(N_CORES))],
                    ins=[arin.opt()], outs=[arout.opt()])
                sq = spool.tile([16, O * B], f32, tag="sq")
                nc.sync.dma_start(sq[:], arout[:])

                # ---------- squash: v = s * |s|^2/(1+|s|^2)/(|s|+1e-8) ----
                s2 = spool.tile([16, O * B], f32, tag="s2t")
                nc.vector.tensor_mul(s2[:], sq[:], sq[:])
                ssq_a = ps_q.tile([1, 320], f32, tag="ssq")
                ssq_b = ps_q.tile([1, 320], f32, tag="ssq")
                nc.tensor.matmul(ssq_a[:], ones16[:], s2[:, 0:320],
                                 start=True, stop=True)
                nc.tensor.matmul(ssq_b[:], ones16[:], s2[:, 320:640],
                                 start=True, stop=True)
                ssq = spool.tile([1, O * B], f32, tag="ssq")
                nc.vector.tensor_scalar(out=ssq[:, 0:320], in0=ssq_a[:],
                                        scalar1=1e-12, scalar2=None,
                                        op0=ALU.add)
                nc.vector.tensor_scalar(out=ssq[:, 320:640], in0=ssq_b[:],
                                        scalar1=1e-12, scalar2=None,
                                        op0=ALU.add)
                # norm = exp(0.5*ln(ssq+eps)) keeps ACT on one table set
                lns = spool.tile([1, O * B], f32, tag="lns")
                nc.scalar.activation(lns[:], ssq[:], AF.Ln)
                nrm = spool.tile([1, O * B], f32, tag="nrm")
                nc.scalar.activation(nrm[:], lns[:], AF.Exp, scale=0.5)
                den = spool.tile([1, O * B], f32, tag="den")
                nc.vector.tensor_scalar(out=den[:], in0=nrm[:], scalar1=1e-8,
                                        scalar2=None, op0=ALU.add)
                onep = spool.tile([1, O * B], f32, tag="onep")
                nc.vector.tensor_scalar(out=onep[:], in0=ssq[:], scalar1=1.0,
                                        scalar2=None, op0=ALU.add)
                nc.vector.tensor_mul(den[:], den[:], onep[:])
                nc.vector.reciprocal(den[:], den[:])
                scl = spool.tile([1, O * B], f32, tag="scl")
                nc.vector.tensor_mul(scl[:], ssq[:], den[:])
                sclb = spool.tile([16, O * B], f32, tag="sclb")
                nc.gpsimd.partition_broadcast(sclb[:], scl[:])
                vt = spool.tile([16, O * B], f32, tag="vt")
                nc.vector.tensor_mul(vt[:], sq[:], sclb[:])

                if k == ITERS - 1:
                    nc.sync.dma_start(vout_d.ap(), vt[:])
                else:
                    # ---------- agreement: ell += sum_e x * (W^T v) ------
                    # weights for the K=d matmuls live on 16 partitions;
                    # streamed per (g, o-chunk) to bound SBUF use, chunks
                    # aligned with the o-pair loop: {0-3, 4-7, 8-9}
                    OCH = [(0, 4), (4, 8), (8, 10)]
                    for g in range(G):
                        wgts = []
                        for (o0, o1) in OCH:
                            wt = wpool.tile([16, E, 4, 128], f32, tag="wgt")
                            nc.sync.dma_start(wt[:, :, 0:o1 - o0, :],
                                              wg_d.ap()[g, :, :, o0:o1, :])
                            wgts.append(wt)
                        y2 = wpool.tile([128, O, E, B], f32, tag="y")
                        for j in range(O // 2):
                            gp = ps_g.tile([128, 1024], f32, tag="gps")
                            for oo in range(2):
                                o = 2 * j + oo
                                ci = o // 4
                                wt = wgts[ci]
                                for e in range(E):
                                    nc.tensor.matmul(
                                        gp[:, oo * 512 + e * 64:
                                           oo * 512 + (e + 1) * 64],
                                        wt[:, e, o - OCH[ci][0], :],
                                        vt[:, o * 64:(o + 1) * 64],
                                        start=(e == 0),  # once per bank
                                        stop=(e == E - 1))
                            nc.vector.tensor_mul(
                                y2[:, 2 * j:2 * j + 2, :, :],
                                gp[:].rearrange("p (oo e b) -> p oo e b",
                                                oo=2, e=E),
                                xi[:, g][:, None, :, :].broadcast_to(
                                    [128, 2, E, B]))
                        # e-reduction tree, in place on y2
                        nc.vector.tensor_add(y2[:, :, 0:4, :],
                                             y2[:, :, 0:4, :],
                                             y2[:, :, 4:8, :])
                        nc.vector.tensor_add(y2[:, :, 0:2, :],
                                             y2[:, :, 0:2, :],
                                             y2[:, :, 2:4, :])
                        nc.vector.tensor_add(y2[:, :, 0, :],
                                             y2[:, :, 0, :],
                                             y2[:, :, 1, :])
                        nc.vector.tensor_add(ell[:, g], ell[:, g],
                                             y2[:, :, 0, :])
                    if DBG and k == ITERS - 2:
                        nc.sync.dma_start(ell_d.ap(), ell[:])

    nc.compile()
    return nc


def _host_prep(x, W):
    x = np.asarray(x, dtype=np.float32)
    W = np.asarray(W, dtype=np.float32)
    in_maps = []
    for c in range(N_CORES):
        lo, hi = c * IC, (c + 1) * IC
        xp = np.zeros((B, ICP, E), np.float32)
        xp[:, :IC] = x[:, lo:hi, :]
        # [128, G, E, B]
        xi = np.ascontiguousarray(
            xp.reshape(B, G, 128, E).transpose(2, 1, 3, 0))
        Wp = np.zeros((O, ICP, D, E), np.float32)
        Wp[:, :IC] = W[:, lo:hi, :, :]
        Wr = Wp.reshape(O, G, 128, D, E)
        # ws: [128, G, E, O, D]
        wsl = np.ascontiguousarray(Wr.transpose(2, 1, 4, 0, 3))
        # wg: [G, 16(D), E, O, 128]
        wgl = np.ascontiguousarray(Wr.transpose(1, 3, 4, 0, 2))
        in_maps.append({"xi": xi, "ws": wsl, "wg": wgl})
    return in_maps


def _get_program():
    if "nc" not in _CACHE:
        _CACHE["nc"] = _build_program()
    return _CACHE["nc"]


def run_on_hw(in_maps, trace=False):
    from concourse.bass_utils import run_bass_kernel_spmd

    nc = _get_program()
    return run_bass_kernel_spmd(nc, in_maps, core_ids=list(range(N_CORES)),
                                trace=trace)


def kernel(x, W):
    in_maps = _host_prep(x, W)
    res = run_on_hw(in_maps)
    vt = np.asarray(res.results[0]["vout"], dtype=np.float32)
    # device layout [D, O*B] -> [B, O, D]
    return np.ascontiguousarray(vt.reshape(D, O, B).transpose(2, 1, 0))


if __name__ == "__main__":
    rng = np.random.default_rng(0)
    x = rng.standard_normal((B, I_FULL, E), dtype=np.float32)
    W = (0.01 * rng.standard_normal((O, I_FULL, D, E))).astype(np.float32)
    v = kernel(x, W)
    print(v.shape, v.dtype, float(np.abs(v).max()))


# revision 30
# speedup vs baseline: 5.2152x; 5.2152x over previous
"""DenseCaps dynamic-routing kernel for 8 Trainium2 NeuronCores.

Sharding: in_caps (I=8000) split 8 ways (1000/core, padded to 1024); the
per-iteration sum over in_caps is completed with a tiny AllReduce of the
partial s tensors ([64,10,16] fp32 = 40KB). Everything is SBUF-resident;
u_hat is never materialized:

  s[b,o,d]   = sum_{i,e} W[o,i,d,e] * (c[b,o,i] * x[b,i,e])   (K=i matmuls,
               moving operand y = c*x built on VectorE in i-partition layout)
  a[b,o,i]   = sum_e x[b,i,e] * g[b,o,i,e],
  g[b,o,i,e] = sum_d W[o,i,d,e] * v[b,o,d]                    (K=d matmuls)

Everything is fp32 (the correctness gate is elementwise-relative,
so flat bf16 noise in the big sums is not tolerable). ITERS=3.
"""

import sys
import numpy as np

sys.path.insert(0, "/opt/trn_rl_repo")

B, O, D, E = 64, 10, 16, 8
I_FULL = 8000
N_CORES = 8
IC = 1000          # in_caps per core
ICP = 1024         # padded
G = ICP // 128     # i-chunks of 128 partitions
ITERS = 3

_CACHE = {}


def _build_program():
    import concourse.bacc as bacc
    import concourse.mybir as mybir
    import concourse.tile as tile

    f32 = mybir.dt.float32
    bf16 = mybir.dt.bfloat16
    AF = mybir.ActivationFunctionType
    ALU = mybir.AluOpType
    AX = mybir.AxisListType

    nc = bacc.Bacc("TRN2", target_bir_lowering=False, debug=False,
                   num_devices=N_CORES)

    xi_d = nc.dram_tensor("xi", [128, G, E, B], f32, kind="ExternalInput")
    ws_d = nc.dram_tensor("ws", [128, G, E, O, D], f32, kind="ExternalInput")
    wg_d = nc.dram_tensor("wg", [G, 16, E, O, 128], f32, kind="ExternalInput")
    vout_d = nc.dram_tensor("vout", [16, O * B], f32, kind="ExternalOutput")
    import os
    DBG = bool(int(os.environ.get("KDBG", "0")))
    ABL = os.environ.get("KABL", "")  # ablation: noag|nodma|nodve|nommg|noy
    if DBG:
        ell_d = nc.dram_tensor("ell_dbg", [128, G, O, B], f32,
                               kind="ExternalOutput")

    with tile.TileContext(nc) as tc:
        with tc.tile_pool(name="const", bufs=1) as cpool, \
             tc.tile_pool(name="work", bufs=2) as wpool, \
             tc.tile_pool(name="small", bufs=1) as spool, \
             tc.tile_pool(name="ps_s", bufs=1, space="PSUM") as ps_s, \
             tc.tile_pool(name="ps_g", bufs=2, space="PSUM") as ps_g, \
             tc.tile_pool(name="ps_q", bufs=2, space="PSUM") as ps_q, \
             tc.tile_pool(name="dram", bufs=2, space="DRAM") as dpool:

            xi = cpool.tile([128, G, E, B], f32)
            ws = cpool.tile([128, G, E, O, D], f32)
            ell = cpool.tile([128, G, O, B], f32)
            ones16 = cpool.tile([16, 1], f32)

            nc.sync.dma_start(xi[:], xi_d.ap())
            nc.sync.dma_start(ws[:], ws_d.ap())
            nc.vector.memset(ell[:], 0.0)
            nc.vector.memset(ones16[:], 1.0)

            for k in range(ITERS):
                # ---------- routing weights c ----------
                if k > 0:
                    # c[b,o,i] = exp(ell)/sum_o exp(ell); |ell| << 1 so no
                    # max-subtraction is needed.
                    eexp = spool.tile([128, G, O, B], f32, tag="eexp")
                    nc.scalar.activation(eexp[:], ell[:], AF.Exp)
                    z = wpool.tile([128, G, B], f32, tag="z")
                    nc.vector.tensor_reduce(
                        z[:], eexp[:].transpose([0, 1, 3, 2]),
                        axis=AX.X, op=ALU.add)
                    nc.vector.reciprocal(z[:], z[:])
                    cc = eexp  # normalize in place: cc = eexp * (1/z)
                    nc.vector.tensor_mul(
                        cc[:], eexp[:],
                        z[:, :, None, :].broadcast_to([128, G, O, B]))

                # ---------- s partials: 640 K=128 matmuls ----------
                s_ps1 = ps_s.tile([16, 512], f32, tag="s1")
                s_ps2 = ps_s.tile([16, 128], f32, tag="s2")
                for g in range(G):
                    if k > 0 and ABL != "noy":
                        y = wpool.tile([128, E, O, B], f32, tag="y")
                        nc.vector.tensor_mul(
                            y[:],
                            cc[:, g][:, None, :, :].broadcast_to([128, E, O, B]),
                            xi[:, g][:, :, None, :].broadcast_to([128, E, O, B]))
                    for e in range(E):
                        for o in range(O):
                            rhs = (y[:, e, o, :] if (k > 0 and ABL != "noy")
                                   else xi[:, g, e, :])
                            out_ps = (s_ps1[:, o * 64:(o + 1) * 64] if o < 8
                                      else s_ps2[:, (o - 8) * 64:(o - 7) * 64])
                            # start clears has_written for the WHOLE bank, so
                            # only the first matmul touching each bank sets it.
                            nc.tensor.matmul(
                                out_ps, ws[:, g, e, o, :], rhs,
                                start=(g == 0 and e == 0 and o in (0, 8)),
                                stop=(g == G - 1 and e == E - 1))

                s_sb = spool.tile([16, O * B], f32, tag="ssb")
                nc.vector.tensor_copy(s_sb[:, 0:512], s_ps1[:])
                nc.vector.tensor_copy(s_sb[:, 512:640], s_ps2[:])
                if k == 0:
                    # c == 0.1 exactly on the first iteration
                    nc.vector.tensor_scalar(
                        out=s_sb[:], in0=s_sb[:], scalar1=0.1, scalar2=None,
                        op0=ALU.mult)

                # ---------- AllReduce over the i-shards ----------
                arin = dpool.tile([16, O * B], f32, tag="arin")
                arout = dpool.tile([16, O * B], f32, tag="arout")
                nc.sync.dma_start(arin[:], s_sb[:])
                nc.gpsimd.collective_compute(
                    "AllReduce", ALU.add,
                    replica_groups=[list(range(N_CORES))],
                    ins=[arin.opt()], outs=[arout.opt()])
                sq = spool.tile([16, O * B], f32, tag="sq")
                nc.sync.dma_start(sq[:], arout[:])

                # ---------- squash: v = s * |s|^2/(1+|s|^2)/(|s|+1e-8) ----
                s2 = spool.tile([16, O * B], f32, tag="s2t")
                nc.vector.tensor_mul(s2[:], sq[:], sq[:])
                ssq_a = ps_q.tile([1, 320], f32, tag="ssq")
                ssq_b = ps_q.tile([1, 320], f32, tag="ssq")
                nc.tensor.matmul(ssq_a[:], ones16[:], s2[:, 0:320],
                                 start=True, stop=True)
                nc.tensor.matmul(ssq_b[:], ones16[:], s2[:, 320:640],
                                 start=True, stop=True)
                ssq = spool.tile([1, O * B], f32, tag="ssq")
                nc.vector.tensor_scalar(out=ssq[:, 0:320], in0=ssq_a[:],
                                        scalar1=1e-12, scalar2=None,
                                        op0=ALU.add)
                nc.vector.tensor_scalar(out=ssq[:, 320:640], in0=ssq_b[:],
                                        scalar1=1e-12, scalar2=None,
                                        op0=ALU.add)
                # norm = exp(0.5*ln(ssq+eps)) keeps ACT on one table set
                lns = spool.tile([1, O * B], f32, tag="lns")
                nc.scalar.activation(lns[:], ssq[:], AF.Ln)
                nrm = spool.tile([1, O * B], f32, tag="nrm")
                nc.scalar.activation(nrm[:], lns[:], AF.Exp, scale=0.5)
                den = spool.tile([1, O * B], f32, tag="den")
                nc.vector.tensor_scalar(out=den[:], in0=nrm[:], scalar1=1e-8,
                                        scalar2=None, op0=ALU.add)
                onep = spool.tile([1, O * B], f32, tag="onep")
                nc.vector.tensor_scalar(out=onep[:], in0=ssq[:], scalar1=1.0,
                                        scalar2=None, op0=ALU.add)
                nc.vector.tensor_mul(den[:], den[:], onep[:])
                nc.vector.reciprocal(den[:], den[:])
                scl = spool.tile([1, O * B], f32, tag="scl")
                nc.vector.tensor_mul(scl[:], ssq[:], den[:])
                sclb = spool.tile([16, O * B], f32, tag="sclb")
                nc.gpsimd.partition_broadcast(sclb[:], scl[:])
                vt = spool.tile([16, O * B], f32, tag="vt")
                nc.vector.tensor_mul(vt[:], sq[:], sclb[:])

                if k == ITERS - 1:
                    nc.sync.dma_start(vout_d.ap(), vt[:])
                elif ABL == "noag":
                    pass
                else:
                    # ---------- agreement: ell += sum_e x * (W^T v) ------
                    # weights for the K=d matmuls live on 16 partitions;
                    # streamed per (g, o-chunk) to bound SBUF use, chunks
                    # aligned with the o-pair loop: {0-3, 4-7, 8-9}
                    OCH = [(0, 4), (4, 8), (8, 10)]
                    for g in range(G):
                        wgts = []
                        for (o0, o1) in OCH:
                            wt = wpool.tile([16, E, 4, 128], f32, tag="wgt")
                            if ABL != "nodma":
                                nc.sync.dma_start(wt[:, :, 0:o1 - o0, :],
                                                  wg_d.ap()[g, :, :, o0:o1, :])
                            wgts.append(wt)
                        y2 = wpool.tile([128, O, E, B], f32, tag="y")
                        for j in range(O // 2):
                            gp = ps_g.tile([128, 1024], f32, tag="gps")
                            for oo in range(2):
                                o = 2 * j + oo
                                ci = o // 4
                                wt = wgts[ci]
                                for e in range(E):
                                    if ABL == "nommg":
                                        continue
                                    nc.tensor.matmul(
                                        gp[:, oo * 512 + e * 64:
                                           oo * 512 + (e + 1) * 64],
                                        wt[:, e, o - OCH[ci][0], :],
                                        vt[:, o * 64:(o + 1) * 64],
                                        start=(e == 0),  # once per bank
                                        stop=(e == E - 1))
                            if ABL != "nodve":
                                nc.vector.tensor_mul(
                                    y2[:, 2 * j:2 * j + 2, :, :],
                                    gp[:].rearrange("p (oo e b) -> p oo e b",
                                                    oo=2, e=E),
                                    xi[:, g][:, None, :, :].broadcast_to(
                                        [128, 2, E, B]))
                        # e-reduction tree, in place on y2
                        nc.vector.tensor_add(y2[:, :, 0:4, :],
                                             y2[:, :, 0:4, :],
                                             y2[:, :, 4:8, :])
                        nc.vector.tensor_add(y2[:, :, 0:2, :],
                                             y2[:, :, 0:2, :],
                                             y2[:, :, 2:4, :])
                        nc.vector.tensor_add(y2[:, :, 0, :],
                                             y2[:, :, 0, :],
                                             y2[:, :, 1, :])
                        nc.vector.tensor_add(ell[:, g], ell[:, g],
                                             y2[:, :, 0, :])
                    if DBG and k == ITERS - 2:
                        nc.sync.dma_start(ell_d.ap(), ell[:])

    nc.compile()
    return nc


def _host_prep(x, W):
    x = np.asarray(x, dtype=np.float32)
    W = np.asarray(W, dtype=np.float32)
    in_maps = []
    for c in range(N_CORES):
        lo, hi = c * IC, (c + 1) * IC
        xp = np.zeros((B, ICP, E), np.float32)
        xp[:, :IC] = x[:, lo:hi, :]
        # [128, G, E, B]
        xi = np.ascontiguousarray(
            xp.reshape(B, G, 128, E).transpose(2, 1, 3, 0))
        Wp = np.zeros((O, ICP, D, E), np.float32)
        Wp[:, :IC] = W[:, lo:hi, :, :]
        Wr = Wp.reshape(O, G, 128, D, E)
        # ws: [128, G, E, O, D]
        wsl = np.ascontiguousarray(Wr.transpose(2, 1, 4, 0, 3))
        # wg: [G, 16(D), E, O, 128]
        wgl = np.ascontiguousarray(Wr.transpose(1, 3, 4, 0, 2))
        in_maps.append({"xi": xi, "ws": wsl, "wg": wgl})
    return in_maps


def _get_program():
    key = ("nc", ITERS)
    if key not in _CACHE:
        _CACHE[key] = _build_program()
    return _CACHE[key]


def run_on_hw(in_maps, trace=False):
    from concourse.bass_utils import run_bass_kernel_spmd

    nc = _get_program()
    return run_bass_kernel_spmd(nc, in_maps, core_ids=list(range(N_CORES)),
                                trace=trace)


def kernel(x, W):
    in_maps = _host_prep(x, W)
    res = run_on_hw(in_maps)
    vt = np.asarray(res.results[0]["vout"], dtype=np.float32)
    # device layout [D, O*B] -> [B, O, D]
    return np.ascontiguousarray(vt.reshape(D, O, B).transpose(2, 1, 0))


if __name__ == "__main__":
    rng = np.random.default_rng(0)
    x = rng.standard_normal((B, I_FULL, E), dtype=np.float32)
    W = (0.01 * rng.standard_normal((O, I_FULL, D, E))).astype(np.float32)
    v = kernel(x, W)
    print(v.shape, v.dtype, float(np.abs(v).max()))


# revision 37
# speedup vs baseline: 5.7598x; 1.1044x over previous
"""DenseCaps dynamic-routing kernel for 8 Trainium2 NeuronCores.

Sharding: in_caps (I=8000) split 8 ways (1000/core, padded to 1024); the
per-iteration sum over in_caps is completed with a tiny AllReduce of the
partial s tensors ([64,10,16] fp32 = 40KB). Everything is SBUF-resident;
u_hat is never materialized:

  s[b,o,d]   = sum_{i,e} W[o,i,d,e] * (c[b,o,i] * x[b,i,e])   (K=i matmuls,
               moving operand y = c*x built on VectorE in i-partition layout)
  a[b,o,i]   = sum_e x[b,i,e] * g[b,o,i,e],
  g[b,o,i,e] = sum_d W[o,i,d,e] * v[b,o,d]                    (K=d matmuls)

Everything is fp32 (the correctness gate is elementwise-relative,
so flat bf16 noise in the big sums is not tolerable). ITERS=3.
"""

import sys
import numpy as np

sys.path.insert(0, "/opt/trn_rl_repo")

B, O, D, E = 64, 10, 16, 8
I_FULL = 8000
N_CORES = 8
IC = 1000          # in_caps per core
ICP = 1024         # padded
G = ICP // 128     # i-chunks of 128 partitions
ITERS = 3

_CACHE = {}


def _build_program():
    import concourse.bacc as bacc
    import concourse.mybir as mybir
    import concourse.tile as tile

    f32 = mybir.dt.float32
    bf16 = mybir.dt.bfloat16
    AF = mybir.ActivationFunctionType
    ALU = mybir.AluOpType
    AX = mybir.AxisListType

    nc = bacc.Bacc("TRN2", target_bir_lowering=False, debug=False,
                   num_devices=N_CORES)

    xi_d = nc.dram_tensor("xi", [128, G, E, B], f32, kind="ExternalInput")
    ws_d = nc.dram_tensor("ws", [128, G, E, O, D], f32, kind="ExternalInput")
    wg_d = nc.dram_tensor("wg", [G, 16, E, O, 128], f32, kind="ExternalInput")
    vout_d = nc.dram_tensor("vout", [16, O * B], f32, kind="ExternalOutput")
    import os
    DBG = bool(int(os.environ.get("KDBG", "0")))
    ABL = os.environ.get("KABL", "")  # ablation: noag|nodma|nodve|nommg|noy
    if DBG:
        ell_d = nc.dram_tensor("ell_dbg", [128, G, O, B], f32,
                               kind="ExternalOutput")

    with tile.TileContext(nc) as tc:
        with tc.tile_pool(name="const", bufs=1) as cpool, \
             tc.tile_pool(name="work", bufs=2) as wpool, \
             tc.tile_pool(name="small", bufs=1) as spool, \
             tc.tile_pool(name="ps_s", bufs=1, space="PSUM") as ps_s, \
             tc.tile_pool(name="ps_g", bufs=2, space="PSUM") as ps_g, \
             tc.tile_pool(name="ps_q", bufs=2, space="PSUM") as ps_q, \
             tc.tile_pool(name="dram", bufs=2, space="DRAM") as dpool:

            xi = cpool.tile([128, G, E, B], f32)
            ws = cpool.tile([128, G, E, O, D], f32)
            ell = cpool.tile([128, G, O, B], f32)
            ones16 = cpool.tile([16, 1], f32)

            nc.sync.dma_start(xi[:], xi_d.ap())
            nc.sync.dma_start(ws[:], ws_d.ap())
            nc.vector.memset(ell[:], 0.0)
            nc.vector.memset(ones16[:], 1.0)

            for k in range(ITERS):
                # ---------- routing weights c ----------
                if k > 0:
                    # c[b,o,i] = exp(ell)/sum_o exp(ell); |ell| << 1 so no
                    # max-subtraction is needed.
                    eexp = spool.tile([128, G, O, B], f32, tag="eexp")
                    nc.scalar.activation(eexp[:], ell[:], AF.Exp)
                    z = wpool.tile([128, G, B], f32, tag="z")
                    nc.vector.tensor_reduce(
                        z[:], eexp[:].transpose([0, 1, 3, 2]),
                        axis=AX.X, op=ALU.add)
                    nc.vector.reciprocal(z[:], z[:])
                    cc = eexp  # normalize in place: cc = eexp * (1/z)
                    nc.vector.tensor_mul(
                        cc[:], eexp[:],
                        z[:, :, None, :].broadcast_to([128, G, O, B]))

                # ---------- s partials: 640 K=128 matmuls ----------
                k0merge = (k == 0)
                if k0merge:
                    s_pm1 = ps_s.tile([128, 64], f32, tag="s1")
                else:
                    s_ps1 = ps_s.tile([16, 512], f32, tag="s1")
                s_ps2 = ps_s.tile([16, 128], f32, tag="s2")
                for g in range(G):
                    if k > 0 and ABL != "noy":
                        y = wpool.tile([128, E, O, B], f32, tag="y")
                        nc.vector.tensor_mul(
                            y[:],
                            cc[:, g][:, None, :, :].broadcast_to([128, E, O, B]),
                            xi[:, g][:, :, None, :].broadcast_to([128, E, O, B]))
                    for e in range(E):
                        if k0merge:
                            # c uniform in pass 0: one M=128 matmul covers
                            # o 0..7 (lhsT = W slabs, contiguous (o,d) cols)
                            rhs = xi[:, g, e, :]
                            nc.tensor.matmul(
                                s_pm1[:],
                                ws[:, g, e, 0:8, :].rearrange(
                                    "p o d -> p (o d)"),
                                rhs,
                                start=(g == 0 and e == 0),
                                stop=(g == G - 1 and e == E - 1))
                            for o in (8, 9):
                                nc.tensor.matmul(
                                    s_ps2[:, (o - 8) * 64:(o - 7) * 64],
                                    ws[:, g, e, o, :], rhs,
                                    start=(g == 0 and e == 0 and o == 8),
                                    stop=(g == G - 1 and e == E - 1))
                            continue
                        for o in range(O):
                            rhs = y[:, e, o, :]
                            out_ps = (s_ps1[:, o * 64:(o + 1) * 64] if o < 8
                                      else s_ps2[:, (o - 8) * 64:(o - 7) * 64])
                            # start clears has_written for the WHOLE bank, so
                            # only the first matmul touching each bank sets it.
                            nc.tensor.matmul(
                                out_ps, ws[:, g, e, o, :], rhs,
                                start=(g == 0 and e == 0 and o in (0, 8)),
                                stop=(g == G - 1 and e == E - 1))

                s_sb = spool.tile([16, O * B], f32, tag="ssb")
                if k0merge:
                    s_st = spool.tile([128, 64], f32, tag="sst")
                    nc.vector.tensor_copy(s_st[:], s_pm1[:])
                    # reshuffle [(o8,d16), b] -> [d16, (o,b)]: strip DMAs
                    for o in range(8):
                        nc.sync.dma_start(
                            s_sb[:, o * 64:(o + 1) * 64],
                            s_st[16 * o:16 * o + 16, :])
                else:
                    nc.vector.tensor_copy(s_sb[:, 0:512], s_ps1[:])
                nc.vector.tensor_copy(s_sb[:, 512:640], s_ps2[:])
                if k == 0:
                    # c == 0.1 exactly on the first iteration
                    nc.vector.tensor_scalar(
                        out=s_sb[:], in0=s_sb[:], scalar1=0.1, scalar2=None,
                        op0=ALU.mult)

                # ---------- AllReduce over the i-shards ----------
                arin = dpool.tile([16, O * B], f32, tag="arin")
                arout = dpool.tile([16, O * B], f32, tag="arout")
                nc.sync.dma_start(arin[:], s_sb[:])
                nc.gpsimd.collective_compute(
                    "AllReduce", ALU.add,
                    replica_groups=[list(range(N_CORES))],
                    ins=[arin.opt()], outs=[arout.opt()])
                sq = spool.tile([16, O * B], f32, tag="sq")
                nc.sync.dma_start(sq[:], arout[:])

                # ---------- squash: v = s * |s|^2/(1+|s|^2)/(|s|+1e-8) ----
                s2 = spool.tile([16, O * B], f32, tag="s2t")
                nc.vector.tensor_mul(s2[:], sq[:], sq[:])
                ssq_a = ps_q.tile([1, 320], f32, tag="ssq")
                ssq_b = ps_q.tile([1, 320], f32, tag="ssq")
                nc.tensor.matmul(ssq_a[:], ones16[:], s2[:, 0:320],
                                 start=True, stop=True)
                nc.tensor.matmul(ssq_b[:], ones16[:], s2[:, 320:640],
                                 start=True, stop=True)
                ssq = spool.tile([1, O * B], f32, tag="ssq")
                nc.vector.tensor_scalar(out=ssq[:, 0:320], in0=ssq_a[:],
                                        scalar1=1e-12, scalar2=None,
                                        op0=ALU.add)
                nc.vector.tensor_scalar(out=ssq[:, 320:640], in0=ssq_b[:],
                                        scalar1=1e-12, scalar2=None,
                                        op0=ALU.add)
                # norm = exp(0.5*ln(ssq+eps)) keeps ACT on one table set
                lns = spool.tile([1, O * B], f32, tag="lns")
                nc.scalar.activation(lns[:], ssq[:], AF.Ln)
                nrm = spool.tile([1, O * B], f32, tag="nrm")
                nc.scalar.activation(nrm[:], lns[:], AF.Exp, scale=0.5)
                den = spool.tile([1, O * B], f32, tag="den")
                nc.vector.tensor_scalar(out=den[:], in0=nrm[:], scalar1=1e-8,
                                        scalar2=None, op0=ALU.add)
                onep = spool.tile([1, O * B], f32, tag="onep")
                nc.vector.tensor_scalar(out=onep[:], in0=ssq[:], scalar1=1.0,
                                        scalar2=None, op0=ALU.add)
                nc.vector.tensor_mul(den[:], den[:], onep[:])
                nc.vector.reciprocal(den[:], den[:])
                scl = spool.tile([1, O * B], f32, tag="scl")
                nc.vector.tensor_mul(scl[:], ssq[:], den[:])
                sclb = spool.tile([16, O * B], f32, tag="sclb")
                nc.gpsimd.partition_broadcast(sclb[:], scl[:])
                vt = spool.tile([16, O * B], f32, tag="vt")
                nc.vector.tensor_mul(vt[:], sq[:], sclb[:])

                if k == ITERS - 1:
                    nc.sync.dma_start(vout_d.ap(), vt[:])
                elif ABL == "noag":
                    pass
                else:
                    # ---------- agreement: ell += sum_e x * (W^T v) ------
                    # weights for the K=d matmuls live on 16 partitions;
                    # streamed per (g, o-chunk) to bound SBUF use, chunks
                    # aligned with the o-pair loop: {0-3, 4-7, 8-9}
                    OCH = [(0, 4), (4, 8), (8, 10)]
                    for g in range(G):
                        wgts = []
                        for (o0, o1) in OCH:
                            wt = wpool.tile([16, E, 4, 128], f32, tag="wgt")
                            if ABL != "nodma":
                                nc.sync.dma_start(wt[:, :, 0:o1 - o0, :],
                                                  wg_d.ap()[g, :, :, o0:o1, :])
                            wgts.append(wt)
                        y2 = wpool.tile([128, O, E, B], f32, tag="y")
                        for j in range(O // 2):
                            gp = ps_g.tile([128, 1024], f32, tag="gps")
                            for oo in range(2):
                                o = 2 * j + oo
                                ci = o // 4
                                wt = wgts[ci]
                                for e in range(E):
                                    if ABL == "nommg":
                                        continue
                                    nc.tensor.matmul(
                                        gp[:, oo * 512 + e * 64:
                                           oo * 512 + (e + 1) * 64],
                                        wt[:, e, o - OCH[ci][0], :],
                                        vt[:, o * 64:(o + 1) * 64],
                                        start=(e == 0),  # once per bank
                                        stop=(e == E - 1))
                            if ABL != "nodve":
                                nc.vector.tensor_mul(
                                    y2[:, 2 * j:2 * j + 2, :, :],
                                    gp[:].rearrange("p (oo e b) -> p oo e b",
                                                    oo=2, e=E),
                                    xi[:, g][:, None, :, :].broadcast_to(
                                        [128, 2, E, B]))
                        # e-reduction: one strided reduce (innermost=e)
                        ag = wpool.tile([128, O, B], f32, tag="ag")
                        nc.vector.tensor_reduce(
                            ag[:], y2[:].transpose([0, 1, 3, 2]),
                            axis=AX.X, op=ALU.add)
                        nc.vector.tensor_add(ell[:, g], ell[:, g], ag[:])
                    if DBG and k == ITERS - 2:
                        nc.sync.dma_start(ell_d.ap(), ell[:])

    nc.compile()
    return nc


def _host_prep(x, W):
    x = np.asarray(x, dtype=np.float32)
    W = np.asarray(W, dtype=np.float32)
    in_maps = []
    for c in range(N_CORES):
        lo, hi = c * IC, (c + 1) * IC
        xp = np.zeros((B, ICP, E), np.float32)
        xp[:, :IC] = x[:, lo:hi, :]
        # [128, G, E, B]
        xi = np.ascontiguousarray(
            xp.reshape(B, G, 128, E).transpose(2, 1, 3, 0))
        Wp = np.zeros((O, ICP, D, E), np.float32)
        Wp[:, :IC] = W[:, lo:hi, :, :]
        Wr = Wp.reshape(O, G, 128, D, E)
        # ws: [128, G, E, O, D]
        wsl = np.ascontiguousarray(Wr.transpose(2, 1, 4, 0, 3))
        # wg: [G, 16(D), E, O, 128]
        wgl = np.ascontiguousarray(Wr.transpose(1, 3, 4, 0, 2))
        in_maps.append({"xi": xi, "ws": wsl, "wg": wgl})
    return in_maps


def _get_program():
    key = ("nc", ITERS)
    if key not in _CACHE:
        _CACHE[key] = _build_program()
    return _CACHE[key]


def run_on_hw(in_maps, trace=False):
    from concourse.bass_utils import run_bass_kernel_spmd

    nc = _get_program()
    return run_bass_kernel_spmd(nc, in_maps, core_ids=list(range(N_CORES)),
                                trace=trace)


def kernel(x, W):
    in_maps = _host_prep(x, W)
    res = run_on_hw(in_maps)
    vt = np.asarray(res.results[0]["vout"], dtype=np.float32)
    # device layout [D, O*B] -> [B, O, D]
    return np.ascontiguousarray(vt.reshape(D, O, B).transpose(2, 1, 0))


if __name__ == "__main__":
    rng = np.random.default_rng(0)
    x = rng.standard_normal((B, I_FULL, E), dtype=np.float32)
    W = (0.01 * rng.standard_normal((O, I_FULL, D, E))).astype(np.float32)
    v = kernel(x, W)
    print(v.shape, v.dtype, float(np.abs(v).max()))
